# revision 1
# baseline (speedup 1.0000x reference)
"""Trainium2 Bass kernel for DepthwiseCorrelation.

Pipeline (per sample):
  t = relu(GN(conv1x1(template, w_t)))            # [64, 14, 14]
  s = relu(GN(conv1x1(search, w_s)))              # [64, 64, 64]
  corr = s * mean(t) + depthwise_corr7x7(s, pool2x2(t))
  y = relu(GN(conv3x3(corr, w_p1)))
  out = conv1x1(y, w_p2) + b_p2                   # [1, 64, 64]

Sharding: data-parallel over batch, 32 samples -> 8 cores x 4 samples.
Each core processes its 4 samples as 2 "pairs": two samples' 64 channels
stacked on the 128 SBUF partitions.  All convs become matmuls with
block-diagonal (per-pair) weights; the depthwise correlation becomes 50
diagonal-matmul taps (1 global + 49 shifted) accumulating in PSUM.
GroupNorm groups are adjacent channel pairs, handled by bn_stats/bn_aggr
plus one tiny pair-combine matmul against a fixed 0/1 averaging matrix.
"""

import os
import numpy as np

import concourse.bass as bass
import concourse.bacc as bacc
import concourse.tile as tile
import concourse.mybir as mybir
from concourse.bass_utils import run_bass_kernel_spmd

N_CORES = 8
B = 32
C = 256          # input channels
CC = 64          # corr channels
HS = WS = 64     # search spatial
HT = WT = 14     # template spatial
SP = HS * WS     # 4096
TSP = HT * WT    # 196
KD = 7           # depthwise kernel
RAD = KD // 2    # 3
EPS = 1e-5
PB = B // N_CORES    # 4 samples per core
NPAIR = PB // 2      # 2 pairs per core
XG = 8               # x-rows per psum group
NG = HS // XG        # 8 groups

F32 = mybir.dt.float32
F32R = mybir.dt.float32r
AX = mybir.AxisListType
ALU = mybir.AluOpType
ACT = mybir.ActivationFunctionType

_CACHE = {}
LAST_RESULTS = None  # BassKernelResults of the most recent kernel() call


def _r(ap):
    """matmul inputs are already float32r-typed"""
    return ap


def build_program():
    nc = bacc.Bacc("TRN2", target_bir_lowering=False, debug=False)

    d_search = nc.dram_tensor("search", [NPAIR, 4, 128, SP], F32R, kind="ExternalInput")
    d_templ = nc.dram_tensor("templ", [NPAIR, 4, 128, TSP], F32R, kind="ExternalInput")
    d_ws = nc.dram_tensor("ws_lhsT", [4, 128, 128], F32R, kind="ExternalInput")
    d_wt = nc.dram_tensor("wt_lhsT", [4, 128, 128], F32R, kind="ExternalInput")
    d_wp1 = nc.dram_tensor("wp1_lhsT", [9, 128, 128], F32R, kind="ExternalInput")
    d_wp2 = nc.dram_tensor("wp2_lhsT", [128, 2], F32R, kind="ExternalInput")
    d_apair = nc.dram_tensor("a_pair", [128, 128], F32, kind="ExternalInput")
    d_ident = nc.dram_tensor("ident", [128, 128], F32, kind="ExternalInput")
    d_gn = nc.dram_tensor("gn_vecs", [128, 6], F32, kind="ExternalInput")
    d_bp2 = nc.dram_tensor("b_p2", [2, 1], F32, kind="ExternalInput")
    d_zeros = nc.dram_tensor("zeros", [128, 4900], F32R, kind="ExternalInput")
    d_out = nc.dram_tensor("out", [NPAIR, 2, SP], F32, kind="ExternalOutput")

    with tile.TileContext(nc) as tc:
        _emit(tc, d_search, d_templ, d_ws, d_wt, d_wp1, d_wp2, d_apair, d_ident,
              d_gn, d_bp2, d_zeros, d_out)
    nc.compile()
    return nc


def _emit(tc, d_search, d_templ, d_ws, d_wt, d_wp1, d_wp2, d_apair, d_ident,
          d_gn, d_bp2, d_zeros, d_out):
    nc = tc.nc
    from contextlib import ExitStack
    ctx = ExitStack()
    with ctx:
        const = ctx.enter_context(tc.tile_pool(name="const", bufs=1))
        small = ctx.enter_context(tc.tile_pool(name="small", bufs=4))
        tchp = ctx.enter_context(tc.tile_pool(name="tch", bufs=2))
        schp = ctx.enter_context(tc.tile_pool(name="sch", bufs=5))
        bigp = ctx.enter_context(tc.tile_pool(name="big", bufs=2))
        diagp = ctx.enter_context(tc.tile_pool(name="diag", bufs=1))
        outp = ctx.enter_context(tc.tile_pool(name="outsb", bufs=3))
        ps_s = ctx.enter_context(tc.tile_pool(name="ps_s", bufs=2, space="PSUM"))
        ps_c = ctx.enter_context(tc.tile_pool(name="ps_c", bufs=2, space="PSUM"))
        ps_y = ctx.enter_context(tc.tile_pool(name="ps_y", bufs=2, space="PSUM"))
        ps_sm = ctx.enter_context(tc.tile_pool(name="ps_sm", bufs=2, space="PSUM"))

        # --- constants -------------------------------------------------
        ws_t = const.tile([128, 4 * 128], F32R)
        wt_t = const.tile([128, 4 * 128], F32R)
        wp1_t = const.tile([128, 9 * 128], F32R)
        wp2_t = const.tile([128, 2], F32R)
        ap_t = const.tile([128, 128], F32)
        id_t = const.tile([128, 128], F32)
        gn_t = const.tile([128, 6], F32)
        bp2_t = const.tile([2, 1], F32)
        eps_t = const.tile([128, 1], F32)
        nc.vector.memset(eps_t[:], EPS)
        for j in range(4):
            nc.sync.dma_start(ws_t[:, bass.ts(j, 128)], d_ws[j])
            nc.sync.dma_start(wt_t[:, bass.ts(j, 128)], d_wt[j])
        for e in range(9):
            nc.sync.dma_start(wp1_t[:, bass.ts(e, 128)], d_wp1[e])
        nc.sync.dma_start(wp2_t[:], d_wp2[:])
        nc.sync.dma_start(ap_t[:], d_apair[:])
        nc.sync.dma_start(id_t[:], d_ident[:])
        nc.sync.dma_start(gn_t[:], d_gn[:])
        nc.sync.dma_start(bp2_t[:], d_bp2[:])

        def gn_combine(stats2, w_col, b_col):
            """stats2: [128,2] sbuf (mean, var) per partition.
            Returns (scale, bias) [128,1] tiles applying GN over partition
            pairs: scale = gn_w * rsqrt(var_g + eps), bias = gn_b - mean_g*scale."""
            comb = ps_sm.tile([128, 2], F32, tag="sm")
            nc.tensor.matmul(comb[:], lhsT=ap_t[:], rhs=stats2[:],
                             start=True, stop=True)
            dm = small.tile([128, 1], F32, tag="dm")
            nc.vector.tensor_tensor(out=dm[:], in0=stats2[:, 0:1],
                                    in1=comb[:, 0:1], op=ALU.subtract)
            varg = small.tile([128, 1], F32, tag="varg")
            nc.vector.scalar_tensor_tensor(out=varg[:], in0=dm[:], scalar=dm[:],
                                           in1=comb[:, 1:2], op0=ALU.mult,
                                           op1=ALU.add)
            std = small.tile([128, 1], F32, tag="std")
            nc.scalar.activation(std[:], varg[:], ACT.Sqrt, bias=eps_t[:])
            rstd = small.tile([128, 1], F32, tag="rstd")
            nc.vector.reciprocal(rstd[:], std[:])
            scale = small.tile([128, 1], F32, tag="scale")
            nc.vector.tensor_tensor(out=scale[:], in0=gn_t[:, w_col:w_col + 1],
                                    in1=rstd[:], op=ALU.mult)
            tmp = small.tile([128, 1], F32, tag="tmpms")
            nc.vector.tensor_tensor(out=tmp[:], in0=comb[:, 0:1], in1=scale[:],
                                    op=ALU.mult)
            bias = small.tile([128, 1], F32, tag="bias")
            nc.vector.tensor_tensor(out=bias[:], in0=gn_t[:, b_col:b_col + 1],
                                    in1=tmp[:], op=ALU.subtract)
            return scale, bias

        dw_taps = [(dy, dx) for dy in range(-RAD, RAD + 1)
                   for dx in range(-RAD, RAD + 1)]
        N_PE_TAPS = 36            # depthwise taps on the PE
        N_DVE_TAPS = 13           # taps on the vector engine
        pe_taps = dw_taps[:N_PE_TAPS]
        dve_taps = dw_taps[N_PE_TAPS:N_PE_TAPS + N_DVE_TAPS]
        assert N_PE_TAPS + N_DVE_TAPS == len(dw_taps)
        c3_taps = [(0, 0)] + [(ey, ex) for ey in (-1, 0, 1) for ex in (-1, 0, 1)
                              if (ey, ex) != (0, 0)]
        HP = HS + 2 * RAD  # 70, zero-padded s
        HC = HS + 2        # 66, zero-padded corr
        st = [{} for _ in range(NPAIR)]  # per-pair tiles

        # ---- phase 0: preallocate padded tiles; zero borders early ----
        def phase0(p):
            # Zero the pad borders with uint32-bitcast DVE memsets:
            # top rows + left(first), bottom rows + right(last), and the
            # fused right|left runs between adjacent interior rows.
            U32 = mybir.dt.uint32
            s_pad = bigp.tile([128, HP * HP], F32R, tag="s")
            sf = s_pad[:].bitcast(U32)
            nc.vector.memset(sf[:, 0:RAD * HP + RAD], 0)
            nc.vector.memset(sf[:, (HP - RAD) * HP - RAD:HP * HP], 0)
            nr = HP - 2 * RAD - 1
            nc.vector.memset(
                sf[:, RAD * HP + HP - RAD:RAD * HP + HP - RAD + nr * HP
                   ].rearrange("q (r c) -> q r c", c=HP)[:, :, 0:2 * RAD], 0)
            spv = s_pad[:].rearrange("q (x y) -> q x y", x=HP)
            corr_pad = bigp.tile([128, HC * HC], F32R, tag="corr")
            cf = corr_pad[:].bitcast(U32)
            nc.vector.memset(cf[:, 0:HC + 1], 0)
            nc.vector.memset(cf[:, (HC - 1) * HC - 1:HC * HC], 0)
            ncr = HC - 3
            nc.vector.memset(
                cf[:, HC + HC - 1:HC + HC - 1 + ncr * HC
                   ].rearrange("q (r c) -> q r c", c=HC)[:, :, 0:2], 0)
            cpv = corr_pad[:].rearrange("q (x y) -> q x y", x=HC)
            st[p].update(s_pad=s_pad, spv=spv, corr_pad=corr_pad, cpv=cpv)

        # ---- phase 1: template branch (tiny) -------------------------
        def phase1(p):
            pt = ps_sm.tile([128, TSP], F32, tag="sm")
            for j in range(4):
                tch = tchp.tile([128, TSP], F32R, tag="tch")
                nc.sync.dma_start(tch[:], d_templ[p, j])
                nc.tensor.matmul(pt[:], lhsT=wt_t[:, bass.ts(j, 128)],
                                 rhs=tch[:], start=(j == 0), stop=(j == 3))
            st6t = small.tile([128, 6], F32, tag="st6t")
            nc.vector.bn_stats(st6t[:], pt[:])
            st2t = small.tile([128, 2], F32, tag="st2t")
            nc.vector.bn_aggr(st2t[:], st6t[:])
            scale_t, bias_t = gn_combine(st2t, 2, 3)
            t_sb = tchp.tile([128, TSP], F32, tag="t_sb")
            tsum = small.tile([128, 1], F32, tag="tsum")
            nc.scalar.activation(t_sb[:], pt[:], ACT.Relu, bias=bias_t[:],
                                 scale=scale_t[:], accum_out=tsum[:])
            # 2x2 avg pool -> 7x7 kernel (sums, scaled into kvec)
            tk = small.tile([128, 49], F32, tag="tk")
            tview = t_sb[:].rearrange("q (ky iy kx ix) -> q ky kx iy ix",
                                      ky=7, iy=2, kx=7, ix=2)
            nc.vector.tensor_reduce(tk[:], tview, axis=AX.XY, op=ALU.add)
            kvec = small.tile([128, 50], F32, tag="kvec")
            nc.scalar.mul(kvec[:, 0:1], tsum[:], 1.0 / TSP)
            nc.scalar.mul(kvec[:, 1:50], tk[:], 0.25)
            # diag weight matrices for the PE taps (kvec cols 1..N_PE_TAPS)
            nd = N_PE_TAPS
            diag = diagp.tile([128, nd, 128], F32R, tag="diag")
            id_b = id_t[:].rearrange("q (a m) -> q a m", a=1).broadcast_to(
                [128, nd, 128])
            kv_b = kvec[:, 1:1 + nd].rearrange("q (t a) -> q t a", a=1).broadcast_to(
                [128, nd, 128])
            nc.vector.tensor_tensor(out=diag[:], in0=id_b, in1=kv_b, op=ALU.mult)
            st[p].update(kvec=kvec, diag=diag)

        # ---- phase 2: search conv1x1 + GN + relu ---------------------
        def phase2(p):
            spv = st[p]["spv"]
            st6s = small.tile([128, 8, 6], F32, tag="st6s")
            for q in range(4):
                chs = []
                for j in range(4):
                    ch = schp.tile([128, 1024], F32R, tag="sch")
                    nc.sync.dma_start(ch[:], d_search[p, j, :, bass.ts(q, 1024)])
                    chs.append(ch)
                for n in range(2):
                    nt = q * 2 + n
                    psn = ps_s.tile([128, 512], F32, tag="ps_s")
                    for j in range(4):
                        nc.tensor.matmul(psn[:], lhsT=ws_t[:, bass.ts(j, 128)],
                                         rhs=chs[j][:, bass.ts(n, 512)],
                                         start=(j == 0), stop=(j == 3))
                    nc.vector.bn_stats(st6s[:, nt, :], psn[:])
                    nc.scalar.copy(
                        spv[:, RAD + XG * nt:RAD + XG * (nt + 1), RAD:RAD + WS],
                        psn[:])
            st2s = small.tile([128, 2], F32, tag="st2s")
            nc.vector.bn_aggr(st2s[:], st6s[:].rearrange("q a b -> q (a b)"))
            scale_s, bias_s = gn_combine(st2s, 0, 1)
            s_in = spv[:, RAD:RAD + HS, RAD:RAD + WS]
            nc.scalar.activation(s_in, s_in, ACT.Relu, bias=bias_s[:],
                                 scale=scale_s[:])

        # ---- phase 3: correlation (PE taps in PSUM, DVE taps in SBUF) -
        def phase3a(p, lo, hi):
            """zero borders + DVE accumulator chain (global init + dve_taps
            [lo:hi]); emit in two parts so the other pair's bn_stats can
            interleave on the vector engine mid-chain."""
            spv, kvec = st[p]["spv"], st[p]["kvec"]
            if lo == 0:
                cdve = bigp.tile([128, SP], F32, tag="cdve")
                cdvev = cdve[:].rearrange("q (x y) -> q x y", x=HS)
                nc.vector.tensor_scalar_mul(
                    cdvev[:], spv[:, RAD:RAD + HS, RAD:RAD + WS], kvec[:, 0:1])
                st[p]["cdve"] = cdve
            cdvev = st[p]["cdve"][:].rearrange("q (x y) -> q x y", x=HS)
            for i, (dy, dx) in list(enumerate(dve_taps))[lo:hi]:
                ti = 1 + N_PE_TAPS + i
                win = spv[:, RAD + dy:RAD + dy + HS, RAD + dx:RAD + dx + WS]
                nc.vector.scalar_tensor_tensor(
                    out=cdvev[:], in0=win, scalar=kvec[:, ti:ti + 1],
                    in1=cdvev[:], op0=ALU.mult, op1=ALU.add)

        def phase3b(p):
            """PE diag taps -> psum; ACT evicts into corr_pad (keeps the
            psum ring off the DVE chain's critical path); DVE then adds
            the accumulator in place."""
            spv, diag = st[p]["spv"], st[p]["diag"]
            cpv, cdve = st[p]["cpv"], st[p]["cdve"]
            for g in range(NG):
                pc = ps_c.tile([128, XG * WS], F32, tag="ps_c")
                for i, (dy, dx) in enumerate(pe_taps):
                    nc.tensor.matmul(
                        pc[:], lhsT=diag[:, i, :],
                        rhs=spv[:, RAD + XG * g + dy:RAD + XG * (g + 1) + dy,
                                 RAD + dx:RAD + WS + dx],
                        start=(i == 0), stop=(i == len(pe_taps) - 1))
                cg = cpv[:, 1 + XG * g:1 + XG * (g + 1), 1:1 + WS]
                nc.scalar.copy(cg, pc[:])
                nc.vector.tensor_tensor(out=cg, in0=cg,
                                        in1=cdve[:, bass.ts(g, XG * WS)],
                                        op=ALU.add)

        # ---- phase 4: conv3x3 + GN + relu ----------------------------
        def phase4(p):
            cpv = st[p]["cpv"]
            y_sb = bigp.tile([128, SP], F32R, tag="y")
            st6y = small.tile([128, 8, 6], F32, tag="st6y")
            for g in range(NG):
                py = ps_y.tile([128, XG * WS], F32, tag="ps_y")
                for i, (ey, ex) in enumerate(c3_taps):
                    e = (ey + 1) * 3 + (ex + 1)
                    nc.tensor.matmul(
                        py[:], lhsT=wp1_t[:, bass.ts(e, 128)],
                        rhs=cpv[:, 1 + XG * g + ey:1 + XG * (g + 1) + ey,
                                 1 + ex:1 + WS + ex],
                        start=(i == 0), stop=(i == len(c3_taps) - 1))
                nc.vector.bn_stats(st6y[:, g, :], py[:])
                nc.scalar.copy(y_sb[:, bass.ts(g, XG * WS)], py[:])
            st2y = small.tile([128, 2], F32, tag="st2y")
            nc.vector.bn_aggr(st2y[:], st6y[:].rearrange("q a b -> q (a b)"))
            scale_y, bias_y = gn_combine(st2y, 4, 5)
            nc.scalar.activation(y_sb[:], y_sb[:], ACT.Relu, bias=bias_y[:],
                                 scale=scale_y[:])
            st[p]["y_sb"] = y_sb

        # ---- phase 5: final 1x1 (-> 1 channel per sample) + bias -----
        def phase5(p):
            y_sb = st[p]["y_sb"]
            for n in range(8):
                po = ps_sm.tile([2, 512], F32, tag="sm")
                nc.tensor.matmul(po[:], lhsT=wp2_t[:],
                                 rhs=y_sb[:, bass.ts(n, 512)],
                                 start=True, stop=True)
                ob = outp.tile([2, 512], F32, tag="out_sb")
                nc.scalar.activation(ob[:], po[:],
                                     ACT.Identity, bias=bp2_t[:], scale=1.0)
                nc.sync.dma_start(d_out[p, :, bass.ts(n, 512)], ob[:])

        # emission order: search pair0 first (hides DMA startup), then
        # templates, then the rest pipelined pair-by-pair
        phase0(0)
        phase0(1)
        phase1(0)
        phase1(1)
        phase2(0)
        phase3a(0, 0, N_DVE_TAPS)
        phase3b(0)
        phase2(1)
        phase3a(1, 0, N_DVE_TAPS)
        phase3b(1)
        phase4(0)
        phase4(1)
        phase5(0)
        phase5(1)


def make_host_inputs(template_feat, search_feat, w_t, gn_t_w, gn_t_b, w_s,
                     gn_s_w, gn_s_b, w_p1, gn_p_w, gn_p_b, w_p2, b_p2):
    """Build the per-core input maps (host-side packing only)."""
    search = np.ascontiguousarray(search_feat, np.float32).reshape(
        N_CORES, NPAIR, 4, 128, SP)
    templ = np.ascontiguousarray(template_feat, np.float32).reshape(
        N_CORES, NPAIR, 4, 128, TSP)

    def stack_lhsT(w):
        out = np.zeros((4, 128, 128), np.float32)
        out[0, :, 0:64] = w[:, 0:128].T
        out[1, :, 0:64] = w[:, 128:256].T
        out[2, :, 64:128] = w[:, 0:128].T
        out[3, :, 64:128] = w[:, 128:256].T
        return out

    ws_lhsT = stack_lhsT(np.asarray(w_s, np.float32))
    wt_lhsT = stack_lhsT(np.asarray(w_t, np.float32))
    wp1 = np.asarray(w_p1, np.float32)
    wp1_lhsT = np.zeros((9, 128, 128), np.float32)
    for e in range(9):
        ky, kx = e // 3, e % 3
        blk = wp1[:, :, ky, kx].T  # [c, o]
        wp1_lhsT[e, 0:64, 0:64] = blk
        wp1_lhsT[e, 64:128, 64:128] = blk
    wp2_lhsT = np.zeros((128, 2), np.float32)
    wp2_lhsT[0:64, 0] = np.asarray(w_p2, np.float32)[0]
    wp2_lhsT[64:128, 1] = np.asarray(w_p2, np.float32)[0]
    a_pair = np.zeros((128, 128), np.float32)
    for r in range(128):
        a_pair[r, (r // 2) * 2] = 0.5
        a_pair[r, (r // 2) * 2 + 1] = 0.5
    ident = np.eye(128, dtype=np.float32)
    gn_vecs = np.stack([
        np.tile(np.asarray(v, np.float32), 2)
        for v in (gn_s_w, gn_s_b, gn_t_w, gn_t_b, gn_p_w, gn_p_b)
    ], axis=1)  # [128, 6]
    b_p2v = np.full((2, 1), np.asarray(b_p2, np.float32)[0], np.float32)

    in_maps = []
    for c in range(N_CORES):
        in_maps.append({
            "search": search[c], "templ": templ[c],
            "ws_lhsT": ws_lhsT, "wt_lhsT": wt_lhsT, "wp1_lhsT": wp1_lhsT,
            "wp2_lhsT": wp2_lhsT, "a_pair": a_pair, "ident": ident,
            "gn_vecs": gn_vecs, "b_p2": b_p2v,
            "zeros": np.zeros((128, 4900), np.float32),
        })
    return in_maps


def kernel(**inputs):
    global LAST_RESULTS
    if "nc" not in _CACHE:
        _CACHE["nc"] = build_program()
    nc = _CACHE["nc"]
    in_maps = make_host_inputs(**inputs)
    trace = bool(int(os.environ.get("KERNEL_PROFILE", "0")))
    res = run_bass_kernel_spmd(nc, in_maps, core_ids=list(range(N_CORES)),
                               trace=trace)
    LAST_RESULTS = res
    out = np.stack([res.results[c]["out"] for c in range(N_CORES)])  # [8,2,2,SP]
    return out.reshape(B, 1, HS, WS).astype(np.float32)



# revision 6
# speedup vs baseline: 1.0411x; 1.0411x over previous
"""Trainium2 Bass kernel for DepthwiseCorrelation (v2, fp16).

Pipeline (per sample):
  t = relu(GN(conv1x1(template, w_t)))            # [64, 14, 14]
  s = relu(GN(conv1x1(search, w_s)))              # [64, 64, 64]
  corr = s * mean(t) + depthwise_corr7x7(s, pool2x2(t))
  y = relu(GN(conv3x3(corr, w_p1)))
  out = conv1x1(y, w_p2) + b_p2                   # [1, 64, 64]

Sharding: data-parallel over batch, 32 samples -> 8 cores x 4 samples.
Each core processes its 4 samples as 2 "pairs": two samples' 64 channels
stacked on the 128 SBUF partitions.

v2 vs baseline (HW-microbenchmarked):
- fp16 data path everywhere: halves DMA, PE matmuls stay 1 cyc/row and
  back-to-back fp16 matmuls measure 216ns/512-col chunk with LDWEIGHTS
  fully hidden (fp32r was ~268 with exposed weight loads).
- Depthwise taps split three ways: 28 odd-dx taps as PE diag matmuls;
  13 even-dx taps "ACT-assisted" (scalar engine does win*k via Copy
  with per-partition scale into a tmp, DVE adds tmp into corr with the
  fp16-packed 2x tensor_tensor); 8 even-dx taps as plain DVE
  scalar_tensor_tensor (no 2x mode exists for STT); the global-mean
  term initializes corr via an ACT scale-copy (free DVE-wise).
- s_pad row stride 72 / left-pad 4 and corr_pad row stride 68 /
  left-pad 2 keep even-dx fp16 windows 4B-aligned for DVE 2x packing.
- PE-tap psum chunks merge into corr via single DVE psum+sbuf adds
  (597ns measured, cheaper than evict+add).
- GpSimd only zeroes pad buffers and builds diag matrices (its SBUF
  port is shared with DVE's second port - measured 3x slowdown when
  both run 2-port ops, so it gets no tap work).
"""

import os
import numpy as np

import concourse.bass as bass
import concourse.bacc as bacc
import concourse.tile as tile
import concourse.mybir as mybir
from concourse.bass_utils import run_bass_kernel_spmd

N_CORES = 8
B = 32
C = 256          # input channels
CC = 64          # corr channels
HS = WS = 64     # search spatial
HT = WT = 14     # template spatial
SP = HS * WS     # 4096
TSP = HT * WT    # 196
KD = 7           # depthwise kernel
RAD = KD // 2    # 3
EPS = 1e-5
PB = B // N_CORES    # 4 samples per core
NPAIR = PB // 2      # 2 pairs per core
XG = 8               # x-rows per psum chunk
NG = HS // XG        # 8 chunks

# padded s: rows 70 (3+64+3), row stride 72, interior cols [4, 68)
SROW = 72
SH = HS + 2 * RAD    # 70
SPAD = SH * SROW     # 5040
SOFF = 4
# padded corr: rows 66 (1+64+1), row stride 68, interior cols [2, 66)
CROW = 68
CH = HS + 2          # 66
CPAD = CH * CROW     # 4488
COFF = 2

F32 = mybir.dt.float32
F16 = mybir.dt.float16
U16 = mybir.dt.uint16
ALU = mybir.AluOpType
ACT = mybir.ActivationFunctionType
AX = mybir.AxisListType

# ---- depthwise tap assignment ------------------------------------------
# kvec columns: 0 = global-mean tap, 1 + (dy+3)*7 + (dx+3) = local taps.
ODD_TAPS = [(dy, dx) for dy in range(-RAD, RAD + 1) for dx in (-3, -1, 1, 3)]
EVEN_TAPS = [(dy, dx) for dy in range(-RAD, RAD + 1) for dx in (-2, 0, 2)]
PE_TAPS = ODD_TAPS                  # 28 odd-dx taps on PE (diag matmuls)
N_ACT_TAPS = 13
ACT_TAPS = EVEN_TAPS[:N_ACT_TAPS]   # ACT mult + DVE add
DVE_TAPS = EVEN_TAPS[N_ACT_TAPS:]   # 8 plain DVE STT taps

_CACHE = {}
LAST_RESULTS = None  # BassKernelResults of the most recent kernel() call


def build_program():
    nc = bacc.Bacc("TRN2", target_bir_lowering=False, debug=False)

    d_search = nc.dram_tensor("search", [NPAIR, 4, 128, SP], F16, kind="ExternalInput")
    d_templ = nc.dram_tensor("templ", [NPAIR, 4, 128, TSP], F16, kind="ExternalInput")
    d_ws = nc.dram_tensor("ws_lhsT", [4, 128, 128], F16, kind="ExternalInput")
    d_wt = nc.dram_tensor("wt_lhsT", [4, 128, 128], F16, kind="ExternalInput")
    d_wp1 = nc.dram_tensor("wp1_lhsT", [9, 128, 128], F16, kind="ExternalInput")
    d_wp2 = nc.dram_tensor("wp2_lhsT", [128, 2], F16, kind="ExternalInput")
    d_apair = nc.dram_tensor("a_pair", [128, 128], F32, kind="ExternalInput")
    d_ident = nc.dram_tensor("ident", [128, 128], F16, kind="ExternalInput")
    d_gn = nc.dram_tensor("gn_vecs", [128, 6], F32, kind="ExternalInput")
    d_bp2 = nc.dram_tensor("b_p2", [2, 1], F32, kind="ExternalInput")
    d_out = nc.dram_tensor("out", [NPAIR, 2, SP], F32, kind="ExternalOutput")

    with tile.TileContext(nc) as tc:
        _emit(tc, d_search, d_templ, d_ws, d_wt, d_wp1, d_wp2, d_apair,
              d_ident, d_gn, d_bp2, d_out)
    nc.compile()
    return nc


def _emit(tc, d_search, d_templ, d_ws, d_wt, d_wp1, d_wp2, d_apair, d_ident,
          d_gn, d_bp2, d_out):
    nc = tc.nc
    from contextlib import ExitStack
    ctx = ExitStack()
    with ctx:
        const = ctx.enter_context(tc.tile_pool(name="const", bufs=1))
        small = ctx.enter_context(tc.tile_pool(name="small", bufs=4))
        tchp = ctx.enter_context(tc.tile_pool(name="tch", bufs=2))
        schp = ctx.enter_context(tc.tile_pool(name="sch", bufs=8))
        bigp = ctx.enter_context(tc.tile_pool(name="big", bufs=2))
        diagp = ctx.enter_context(tc.tile_pool(name="diag", bufs=1))
        outp = ctx.enter_context(tc.tile_pool(name="outsb", bufs=3))
        tmpp = ctx.enter_context(tc.tile_pool(name="tmpz", bufs=4))
        ps_a = ctx.enter_context(tc.tile_pool(name="ps_a", bufs=2, space="PSUM"))
        ps_sm = ctx.enter_context(tc.tile_pool(name="ps_sm", bufs=2, space="PSUM"))
        ps_dw = ctx.enter_context(tc.tile_pool(name="ps_dw", bufs=2, space="PSUM"))
        ps_y = ctx.enter_context(tc.tile_pool(name="ps_y", bufs=2, space="PSUM"))

        # --- constants -------------------------------------------------
        ws_t = const.tile([128, 4 * 128], F16)
        wt_t = const.tile([128, 4 * 128], F16)
        wp1_t = const.tile([128, 9 * 128], F16)
        wp2_t = const.tile([128, 2], F16)
        ap_t = const.tile([128, 128], F32)
        id_t = const.tile([128, 128], F16)
        gn_t = const.tile([128, 6], F32)
        bp2_t = const.tile([2, 1], F32)
        eps_t = const.tile([128, 1], F32)
        nc.vector.memset(eps_t[:], EPS)
        for j in range(4):
            nc.sync.dma_start(ws_t[:, bass.ts(j, 128)], d_ws[j])
            nc.sync.dma_start(wt_t[:, bass.ts(j, 128)], d_wt[j])
        for e in range(9):
            nc.sync.dma_start(wp1_t[:, bass.ts(e, 128)], d_wp1[e])
        nc.sync.dma_start(wp2_t[:], d_wp2[:])
        nc.sync.dma_start(ap_t[:], d_apair[:])
        nc.sync.dma_start(id_t[:], d_ident[:])
        nc.sync.dma_start(gn_t[:], d_gn[:])
        nc.sync.dma_start(bp2_t[:], d_bp2[:])

        def gn_combine(stats2, w_col, b_col):
            """stats2: [128,2] sbuf (mean, var) per partition. Returns
            (scale, bias) [128,1] applying GN over partition pairs."""
            comb = ps_sm.tile([128, 2], F32, tag="sm")
            nc.tensor.matmul(comb[:], lhsT=ap_t[:], rhs=stats2[:],
                             start=True, stop=True)
            dm = small.tile([128, 1], F32, tag="dm")
            nc.vector.tensor_tensor(out=dm[:], in0=stats2[:, 0:1],
                                    in1=comb[:, 0:1], op=ALU.subtract)
            varg = small.tile([128, 1], F32, tag="varg")
            nc.vector.scalar_tensor_tensor(out=varg[:], in0=dm[:], scalar=dm[:],
                                           in1=comb[:, 1:2], op0=ALU.mult,
                                           op1=ALU.add)
            std = small.tile([128, 1], F32, tag="std")
            nc.scalar.activation(std[:], varg[:], ACT.Sqrt, bias=eps_t[:])
            rstd = small.tile([128, 1], F32, tag="rstd")
            nc.vector.reciprocal(rstd[:], std[:])
            scale = small.tile([128, 1], F32, tag="scale")
            nc.vector.tensor_tensor(out=scale[:], in0=gn_t[:, w_col:w_col + 1],
                                    in1=rstd[:], op=ALU.mult)
            tmp = small.tile([128, 1], F32, tag="tmpms")
            nc.vector.tensor_tensor(out=tmp[:], in0=comb[:, 0:1], in1=scale[:],
                                    op=ALU.mult)
            bias = small.tile([128, 1], F32, tag="bias")
            nc.vector.tensor_tensor(out=bias[:], in0=gn_t[:, b_col:b_col + 1],
                                    in1=tmp[:], op=ALU.subtract)
            return scale, bias

        c3_taps = [(0, 0)] + [(ey, ex) for ey in (-1, 0, 1) for ex in (-1, 0, 1)
                              if (ey, ex) != (0, 0)]
        st = [{} for _ in range(NPAIR)]  # per-pair tiles

        def kcol(dy, dx):
            return 1 + (dy + RAD) * KD + (dx + RAD)

        # ---- phase 0: allocate padded tiles; gpsimd zeroes them -------
        def phase0(p):
            s_pad = bigp.tile([128, SPAD], F16, tag="s", name=f"spad{p}")
            corr_pad = bigp.tile([128, CPAD], F16, tag="corr", name=f"cpad{p}")
            nc.gpsimd.memset(s_pad[:].bitcast(U16), 0)
            nc.gpsimd.memset(corr_pad[:].bitcast(U16), 0)
            spv = s_pad[:].rearrange("q (x y) -> q x y", x=SH)
            cpv = corr_pad[:].rearrange("q (x y) -> q x y", x=CH)
            st[p].update(s_pad=s_pad, spv=spv, corr_pad=corr_pad, cpv=cpv)

        # ---- phase 1: template branch (tiny) -------------------------
        def phase1(p):
            pt = ps_sm.tile([128, TSP], F32, tag="sm", name="ptm")
            for j in range(4):
                tch = tchp.tile([128, TSP], F16, tag="tch")
                nc.sync.dma_start(tch[:], d_templ[p, j])
                nc.tensor.matmul(pt[:], lhsT=wt_t[:, bass.ts(j, 128)],
                                 rhs=tch[:], start=(j == 0), stop=(j == 3))
            st6t = small.tile([128, 6], F32, tag="st6t")
            nc.vector.bn_stats(st6t[:], pt[:])
            st2t = small.tile([128, 2], F32, tag="st2t")
            nc.vector.bn_aggr(st2t[:], st6t[:])
            scale_t, bias_t = gn_combine(st2t, 2, 3)
            t_sb = tchp.tile([128, TSP], F32, tag="t_sb")
            tsum = small.tile([128, 1], F32, tag="tsum")
            nc.scalar.activation(t_sb[:], pt[:], ACT.Relu, bias=bias_t[:],
                                 scale=scale_t[:], accum_out=tsum[:])
            # 2x2 avg pool -> 7x7 kernel; kvec col 0 = global mean
            tk = small.tile([128, 49], F32, tag="tk")
            tview = t_sb[:].rearrange("q (ky iy kx ix) -> q ky kx iy ix",
                                      ky=7, iy=2, kx=7, ix=2)
            nc.vector.tensor_reduce(tk[:], tview, axis=AX.XY, op=ALU.add)
            kvec = small.tile([128, 50], F32, tag="kvec")
            nc.scalar.mul(kvec[:, 0:1], tsum[:], 1.0 / TSP)
            nc.scalar.mul(kvec[:, 1:50], tk[:], 0.25)
            # fp16 diag weights for the 28 odd-dx PE taps, built on gpsimd:
            # diag[c, (dy,dxo), o] = kvec[c, kcol(dy,dx)] * (c == o)
            kv16 = small.tile([128, 50], F16, tag="kv16")
            nc.scalar.copy(kv16[:], kvec[:])
            kv_odd = kv16[:, 1:50].rearrange("q (dy dx) -> q dy dx",
                                             dy=7)[:, :, 0::2]       # [q,7,4]
            diag = diagp.tile([128, 28, 128], F16, tag="diag",
                              name=f"diag{p}")
            dg4 = diag[:].rearrange("q (a b) m -> q a b m", a=7)
            id_b = id_t[:].rearrange("q (a b m) -> q a b m", a=1,
                                     b=1).broadcast_to([128, 7, 4, 128])
            kv_b = kv_odd.rearrange("q a (b c) -> q a b c",
                                    c=1).broadcast_to([128, 7, 4, 128])
            nc.gpsimd.tensor_tensor(out=dg4, in0=id_b, in1=kv_b, op=ALU.mult)
            st[p].update(kvec=kvec, diag=diag)

        # ---- phase 2: search conv1x1 -> psum -> evict (pre-GN) -------
        def phase2(p):
            spv = st[p]["spv"]
            st6s = small.tile([128, 8, 6], F32, tag="st6s", name=f"st6s{p}")
            for q in range(4):
                chs = []
                for j in range(4):
                    ch = schp.tile([128, 1024], F16, tag="sch")
                    nc.sync.dma_start(ch[:], d_search[p, j, :, bass.ts(q, 1024)])
                    chs.append(ch)
                for n in range(2):
                    nt = q * 2 + n
                    psn = ps_a.tile([128, 512], F32, tag="ps_s")
                    for j in range(4):
                        nc.tensor.matmul(psn[:], lhsT=ws_t[:, bass.ts(j, 128)],
                                         rhs=chs[j][:, bass.ts(n, 512)],
                                         start=(j == 0), stop=(j == 3))
                    nc.vector.bn_stats(st6s[:, nt, :], psn[:])
                    nc.scalar.copy(
                        spv[:, RAD + XG * nt:RAD + XG * (nt + 1),
                            SOFF:SOFF + WS], psn[:])
            st[p]["st6s"] = st6s

        # ---- phase 2b: GN combine + relu for s -----------------------
        def phase2b(p):
            spv = st[p]["spv"]
            st2s = small.tile([128, 2], F32, tag="st2s")
            nc.vector.bn_aggr(st2s[:], st[p]["st6s"][:].rearrange(
                "q a b -> q (a b)"))
            scale_s, bias_s = gn_combine(st2s, 0, 1)
            s_in = spv[:, RAD:RAD + HS, SOFF:SOFF + WS]
            nc.scalar.activation(s_in, s_in, ACT.Relu, bias=bias_s[:],
                                 scale=scale_s[:])

        # ---- phase 3: depthwise taps ---------------------------------
        def swin(p, dy, dx, r0, r1):
            return st[p]["spv"][:, RAD + r0 + dy:RAD + r1 + dy,
                                SOFF + dx:SOFF + dx + WS]

        def cwin(p, r0, r1):
            return st[p]["cpv"][:, 1 + r0:1 + r1, COFF:COFF + WS]

        def corr_init(p, h):
            """ACT: corr interior rows of half h = s * global_mean."""
            kvec = st[p]["kvec"]
            r0, r1 = 32 * h, 32 * (h + 1)
            nc.scalar.activation(cwin(p, r0, r1), swin(p, 0, 0, r0, r1),
                                 ACT.Identity, scale=kvec[:, 0:1])

        def act_tap(p, h, dy, dx):
            """ACT computes win*k into a tmp tile; returns it (fp16)."""
            kvec = st[p]["kvec"]
            r0, r1 = 32 * h, 32 * (h + 1)
            tmpz = tmpp.tile([128, 32 * WS], F16, tag="tmpz")
            nc.scalar.activation(
                tmpz[:].rearrange("q (x y) -> q x y", x=32),
                swin(p, dy, dx, r0, r1), ACT.Identity,
                scale=kvec[:, kcol(dy, dx):kcol(dy, dx) + 1])
            return tmpz

        def dve_add_tap(p, h, tmpz):
            r0, r1 = 32 * h, 32 * (h + 1)
            cw = cwin(p, r0, r1)
            nc.vector.tensor_tensor(
                out=cw, in0=tmpz[:].rearrange("q (x y) -> q x y", x=32),
                in1=cw, op=ALU.add)

        def dve_tap(p, h, dy, dx):
            kvec = st[p]["kvec"]
            r0, r1 = 32 * h, 32 * (h + 1)
            cw = cwin(p, r0, r1)
            nc.vector.scalar_tensor_tensor(
                out=cw, in0=swin(p, dy, dx, r0, r1),
                scalar=kvec[:, kcol(dy, dx):kcol(dy, dx) + 1],
                in1=cw, op0=ALU.mult, op1=ALU.add)

        def dve_chain(p, h):
            """All non-PE taps for half h: ACT-assisted adds + STT solos."""
            for i, (dy, dx) in enumerate(ACT_TAPS):
                tmpz = act_tap(p, h, dy, dx)
                dve_add_tap(p, h, tmpz)
                if i < len(DVE_TAPS):
                    dve_tap(p, h, *DVE_TAPS[i])
            for (dy, dx) in DVE_TAPS[len(ACT_TAPS):]:
                dve_tap(p, h, dy, dx)

        def pe_taps(p, g):
            """PE diag taps for chunk g (chunk-outer; LDWEIGHTS hides)."""
            diag = st[p]["diag"]
            pc = ps_dw.tile([128, 512], F32, tag="ps_dw", name=f"dw{p}{g}")
            for i, (dy, dx) in enumerate(PE_TAPS):
                nc.tensor.matmul(
                    pc[:], lhsT=diag[:, i, :],
                    rhs=swin(p, dy, dx, XG * g, XG * (g + 1)),
                    start=(i == 0), stop=(i == len(PE_TAPS) - 1))
            st[p][f"pc{g}"] = pc

        def merge(p, g):
            """corr chunk g += PE psum taps (single DVE psum+sbuf add)."""
            cg = cwin(p, XG * g, XG * (g + 1))
            pc = st[p][f"pc{g}"]
            nc.vector.tensor_tensor(
                out=cg, in0=pc[:].rearrange("q (x y) -> q x y", x=XG),
                in1=cg, op=ALU.add)

        # ---- phase 4: conv3x3 + GN + relu ----------------------------
        def phase4(p, g):
            cpv = st[p]["cpv"]
            if g == 0:
                st[p]["y_sb"] = bigp.tile([128, SP], F16, tag="y",
                                          name=f"ysb{p}")
                st[p]["st6y"] = small.tile([128, 8, 6], F32, tag="st6y",
                                           name=f"st6y{p}")
            y_sb, st6y = st[p]["y_sb"], st[p]["st6y"]
            py = ps_y.tile([128, 512], F32, tag="ps_y", name=f"y{p}{g}")
            for i, (ey, ex) in enumerate(c3_taps):
                e = (ey + 1) * 3 + (ex + 1)
                nc.tensor.matmul(
                    py[:], lhsT=wp1_t[:, bass.ts(e, 128)],
                    rhs=cpv[:, 1 + XG * g + ey:1 + XG * (g + 1) + ey,
                             COFF + ex:COFF + ex + WS],
                    start=(i == 0), stop=(i == len(c3_taps) - 1))
            nc.vector.bn_stats(st6y[:, g, :], py[:])
            nc.scalar.copy(y_sb[:, bass.ts(g, 512)], py[:])

        def phase4b(p):
            y_sb = st[p]["y_sb"]
            st2y = small.tile([128, 2], F32, tag="st2y")
            nc.vector.bn_aggr(st2y[:], st[p]["st6y"][:].rearrange(
                "q a b -> q (a b)"))
            scale_y, bias_y = gn_combine(st2y, 4, 5)
            nc.scalar.activation(y_sb[:], y_sb[:], ACT.Relu, bias=bias_y[:],
                                 scale=scale_y[:])

        # ---- phase 5: final 1x1 (-> 1 channel per sample) + bias -----
        def phase5(p):
            y_sb = st[p]["y_sb"]
            for n in range(8):
                po = ps_sm.tile([2, 512], F32, tag="sm", name="po")
                nc.tensor.matmul(po[:], lhsT=wp2_t[:],
                                 rhs=y_sb[:, bass.ts(n, 512)],
                                 start=True, stop=True)
                ob = outp.tile([2, 512], F32, tag="out_sb")
                nc.scalar.activation(ob[:], po[:],
                                     ACT.Identity, bias=bp2_t[:], scale=1.0)
                nc.sync.dma_start(d_out[p, :, bass.ts(n, 512)], ob[:])

        # ---- emission order (biases the schedule) --------------------
        phase0(0)
        phase0(1)
        phase1(0)
        phase1(1)
        phase2(0)
        phase2b(0)
        corr_init(0, 0)
        corr_init(0, 1)
        for g in range(4):
            pe_taps(0, g)
        dve_chain(0, 0)
        phase2(1)              # PE conv p1 streams behind p0 taps
        phase2b(1)
        for g in range(4, 8):
            pe_taps(0, g)
        dve_chain(0, 1)
        corr_init(1, 0)
        corr_init(1, 1)
        for g in range(4):
            merge(0, g)
        for g in range(4):
            pe_taps(1, g)
        dve_chain(1, 0)
        for g in range(4, 8):
            merge(0, g)
        for g in range(4):
            phase4(0, g)
        for g in range(4, 8):
            pe_taps(1, g)
        dve_chain(1, 1)
        for g in range(4, 8):
            phase4(0, g)
        for g in range(4):
            merge(1, g)
        phase4b(0)
        phase5(0)
        for g in range(4, 8):
            merge(1, g)
        for g in range(8):
            phase4(1, g)
        phase4b(1)
        phase5(1)


def make_host_inputs(template_feat, search_feat, w_t, gn_t_w, gn_t_b, w_s,
                     gn_s_w, gn_s_b, w_p1, gn_p_w, gn_p_b, w_p2, b_p2):
    """Build the per-core input maps (host-side packing only)."""
    search = np.ascontiguousarray(search_feat, np.float16).reshape(
        N_CORES, NPAIR, 4, 128, SP)
    templ = np.ascontiguousarray(template_feat, np.float16).reshape(
        N_CORES, NPAIR, 4, 128, TSP)

    def stack_lhsT(w):
        out = np.zeros((4, 128, 128), np.float16)
        out[0, :, 0:64] = w[:, 0:128].T
        out[1, :, 0:64] = w[:, 128:256].T
        out[2, :, 64:128] = w[:, 0:128].T
        out[3, :, 64:128] = w[:, 128:256].T
        return out

    ws_lhsT = stack_lhsT(np.asarray(w_s, np.float32))
    wt_lhsT = stack_lhsT(np.asarray(w_t, np.float32))
    wp1 = np.asarray(w_p1, np.float32)
    wp1_lhsT = np.zeros((9, 128, 128), np.float16)
    for e in range(9):
        ky, kx = e // 3, e % 3
        blk = wp1[:, :, ky, kx].T  # [c, o]
        wp1_lhsT[e, 0:64, 0:64] = blk
        wp1_lhsT[e, 64:128, 64:128] = blk
    wp2_lhsT = np.zeros((128, 2), np.float16)
    wp2_lhsT[0:64, 0] = np.asarray(w_p2, np.float32)[0]
    wp2_lhsT[64:128, 1] = np.asarray(w_p2, np.float32)[0]
    a_pair = np.zeros((128, 128), np.float32)
    for r in range(128):
        a_pair[r, (r // 2) * 2] = 0.5
        a_pair[r, (r // 2) * 2 + 1] = 0.5
    ident = np.eye(128, dtype=np.float16)
    gn_vecs = np.stack([
        np.tile(np.asarray(v, np.float32), 2)
        for v in (gn_s_w, gn_s_b, gn_t_w, gn_t_b, gn_p_w, gn_p_b)
    ], axis=1)  # [128, 6]
    b_p2v = np.full((2, 1), np.asarray(b_p2, np.float32)[0], np.float32)

    in_maps = []
    for c in range(N_CORES):
        in_maps.append({
            "search": search[c], "templ": templ[c],
            "ws_lhsT": ws_lhsT, "wt_lhsT": wt_lhsT, "wp1_lhsT": wp1_lhsT,
            "wp2_lhsT": wp2_lhsT, "a_pair": a_pair, "ident": ident,
            "gn_vecs": gn_vecs, "b_p2": b_p2v,
        })
    return in_maps


def kernel(**inputs):
    global LAST_RESULTS
    if "nc" not in _CACHE:
        _CACHE["nc"] = build_program()
    nc = _CACHE["nc"]
    in_maps = make_host_inputs(**inputs)
    trace = bool(int(os.environ.get("KERNEL_PROFILE", "0")))
    res = run_bass_kernel_spmd(nc, in_maps, core_ids=list(range(N_CORES)),
                               trace=trace)
    LAST_RESULTS = res
    out = np.stack([res.results[c]["out"] for c in range(N_CORES)])  # [8,2,2,SP]
    return out.reshape(B, 1, HS, WS).astype(np.float32)
